# revision 13
# baseline (speedup 1.0000x reference)
"""Multi-head self-attention on 8 TRN2 NeuronCores.

Strategy: tensor-parallel over heads. Each core owns 2 of the 16 heads
(a 128-column slice of Wq/Wk/Wv and the matching 128-row slice of Wo) and
computes, for both batch elements, its QKV projection columns (transposed
[cols, tokens] layout), full attention for its (batch, head) pairs, and a
partial output projection. The host sums the 8 partial outputs (the
tensor-parallel all-reduce) and adds the output bias.

v2 changes vs the 441us baseline (which was PE-bound at 88% busy and ran
the whole attention phase HAM-throttled at 1.2 GHz):
  - every matmul is fp16 (the old f32r QKV/outproj lowered to
    fp32_mode=HIGH at 2 cycles/column),
  - V is transposed by the DMA xbar (dma_start_transpose) instead of the
    PE+DVE phase whose ~55% PE duty re-throttled the HAM clock gate,
  - scores for the 2 heads run concurrently on disjoint PE row groups,
  - the softmax denominator rides as a 1-column ones-matmul on a spare PE
    column group, concurrent with the AV matmul (tile_position col tiling),
  - exp runs as one FD=1024 ACTIVATE per key tile (both heads),
  - reciprocal_approx_fast instead of the 8-cycle/elem iterative divide,
  - batch-1 QKV projection and the output projections are emitted at
    query-chunk boundaries as dense PE filler that hides the softmax
    normalizer's DRAM broadcast roundtrip.
"""

import os

import numpy as np

# crash-bisection toggles (default = full kernel)
K_TEST = set(os.environ.get("K_TEST", "").split(","))

B, S, D, H, DK = 2, 2048, 1024, 16, 64
NCORES = 8
HPC = H // NCORES          # heads per core = 2
COLS = HPC * DK            # feature columns per core = 128
T = B * S                  # 4096 tokens
TCH = 512                  # query chunk (fp32 PSUM bank width)
PCH = 1024                 # phase-1 token chunk (f16 moving free dim)
ND = D // 128              # 8 contraction chunks
NE = D // 128              # 8 output-column chunks
NQ = S // TCH              # 4 query chunks per batch
NKT = S // 128             # 16 key tiles per batch
NPC = S // PCH             # 2 phase-1 chunks per batch

_CACHE = {}


def _build_program():
    from contextlib import ExitStack

    import concourse.bacc as bacc
    import concourse.mybir as mybir
    import concourse.tile as tile

    f32 = mybir.dt.float32
    f16 = mybir.dt.float16
    Exp = mybir.ActivationFunctionType.Exp

    nc = bacc.Bacc("TRN2", target_bir_lowering=False, debug=False,
                   num_devices=NCORES)

    XT = nc.dram_tensor("xt", [D, T], f16, kind="ExternalInput").ap()
    Wq = nc.dram_tensor("wq", [D, COLS], f16, kind="ExternalInput").ap()
    Wk = nc.dram_tensor("wk", [D, COLS], f16, kind="ExternalInput").ap()
    Wv = nc.dram_tensor("wv", [D, COLS], f16, kind="ExternalInput").ap()
    Bq = nc.dram_tensor("bq", [COLS, 1], f32, kind="ExternalInput").ap()
    Bk = nc.dram_tensor("bk", [COLS, 1], f32, kind="ExternalInput").ap()
    Bv = nc.dram_tensor("bv", [COLS, 1], f32, kind="ExternalInput").ap()
    Wo = nc.dram_tensor("wo", [COLS, D], f16, kind="ExternalInput").ap()
    OT = nc.dram_tensor("ot", [D, T], f16, kind="ExternalOutput").ap()

    import concourse.bass as bass

    with tile.TileContext(nc) as tc, ExitStack() as ctx:
        consts = ctx.enter_context(tc.tile_pool(name="consts", bufs=1))
        xtp = ctx.enter_context(tc.tile_pool(name="xtp", bufs=16))
        big = ctx.enter_context(tc.tile_pool(name="big", bufs=1))
        esbp = ctx.enter_context(tc.tile_pool(name="esbp", bufs=3))
        attnp = ctx.enter_context(tc.tile_pool(name="attnp", bufs=3))
        rp = ctx.enter_context(tc.tile_pool(name="rp", bufs=2))
        rbp = ctx.enter_context(tc.tile_pool(name="rbp", bufs=4))
        obp = ctx.enter_context(tc.tile_pool(name="obp", bufs=4))
        drp = ctx.enter_context(tc.tile_pool(name="drp", bufs=4, space="DRAM"))
        # PSUM: tag "s" [128,1024]x2 = 4 banks, tag "pacc" [128,512]x2 = 2,
        # tag "z" [128,512]x2 = 2 (shared by z accumulators and outproj).
        ps_s = ctx.enter_context(tc.tile_pool(name="ps_s", bufs=2, space="PSUM"))
        ps_p = ctx.enter_context(tc.tile_pool(name="ps_p", bufs=2, space="PSUM"))
        ps_z = ctx.enter_context(tc.tile_pool(name="ps_z", bufs=2, space="PSUM"))

        # ---- constants ----
        w_sb = {}
        b_sb = {}
        for nm, src, bsrc in (("wk", Wk, Bk), ("wv", Wv, Bv), ("wq", Wq, Bq)):
            w_sb[nm] = []
            for d in range(ND):
                wt = consts.tile([128, COLS], f16, name=f"{nm}_{d}")
                nc.sync.dma_start(wt, src[d * 128:(d + 1) * 128, :])
                w_sb[nm].append(wt)
            bt = consts.tile([COLS, 1], f32, name=f"b{nm}_sb")
            nc.sync.dma_start(bt, bsrc)
            b_sb[nm] = bt
        wo_sb = []
        for e in range(NE):
            wt = consts.tile([128, 128], f16, name=f"wo_{e}")
            nc.sync.dma_start(wt, Wo[:, e * 128:(e + 1) * 128])
            wo_sb.append(wt)
        onecol = consts.tile([128, 1], f16, name="onecol")
        nc.vector.memset(onecol, 1.0)
        # pre-load the exp table set so the ~2.7us ACT_TABLE_LOAD happens
        # during phase 1 instead of at the first real exp
        dummy = consts.tile([128, 1], f16, name="dummy")
        nc.scalar.activation(dummy, onecol, Exp)

        QT = big.tile([COLS, T], f16, name="QT")
        KT = big.tile([COLS, T], f16, name="KT")
        proj_out = {"wq": QT, "wk": KT}
        # V goes to DRAM and comes back transposed through the DMA xbar
        # (the xbar transpose path is only validated with DRAM sources)
        vtd = drp.tile([COLS, T], f16, tag="vtd", bufs=1, name="vtd")
        vp = [[[None] * NKT for _ in range(HPC)] for _ in range(B)]
        for b in range(B):
            for h in range(HPC):
                for kt in range(NKT):
                    vp[b][h][kt] = consts.tile([128, DK], f16,
                                               name=f"vp_{b}_{h}_{kt}")

        xts = {}

        def emit_x_dma(b, t):
            ts = slice(b * S + t * PCH, b * S + (t + 1) * PCH)
            cur = []
            for d in range(ND):
                xt_t = xtp.tile([128, PCH], f16, name=f"xt_{b}_{t}_{d}",
                                tag="xt")
                nc.gpsimd.dma_start(xt_t, XT[d * 128:(d + 1) * 128, ts])
                cur.append(xt_t)
            xts[(b, t)] = cur

        def emit_proj_unit(b, t, nm):
            """One phase-1 unit: [128, PCH] of one projection for batch b."""
            ts = slice(b * S + t * PCH, b * S + (t + 1) * PCH)
            acc = ps_s.tile([128, PCH], f32, tag="s", name=f"acc_{b}_{t}_{nm}")
            for half in range(2):
                hs = slice(half * TCH, (half + 1) * TCH)
                for d in range(ND):
                    nc.tensor.matmul(acc[:, hs], lhsT=w_sb[nm][d],
                                     rhs=xts[(b, t)][d][:, hs],
                                     start=(d == 0), stop=(d == ND - 1))
            if nm == "wv":
                vt = big.tile([COLS, PCH], f16, tag="vt", bufs=2,
                              name=f"vt_{b}_{t}")
                nc.vector.tensor_scalar_add(vt, acc, b_sb[nm])
                nc.sync.dma_start(vtd[:, ts], vt)
            else:
                nc.vector.tensor_scalar_add(proj_out[nm][:, ts], acc, b_sb[nm])

        def emit_vp_transposes(b, t):
            """DMA-xbar transposes of V token tiles for batch b, chunk t."""
            for kt in range(t * PCH // 128, (t + 1) * PCH // 128):
                ks = slice(b * S + kt * 128, b * S + (kt + 1) * 128)
                for h in range(HPC):
                    nc.scalar.dma_start_transpose(
                        vp[b][h][kt], vtd[h * DK:(h + 1) * DK, ks])

        # ---- phase 1 for batch 0 ----
        # K first (scores need the full key range), then V chunk 0 (vp tiles
        # for the first AV matmuls), then Q chunk 0 (first scores) — gets the
        # scalar engine streaming exps as early as possible.
        for t in range(NPC):
            emit_x_dma(0, t)
        emit_proj_unit(0, 0, "wk")
        emit_proj_unit(0, 1, "wk")
        emit_proj_unit(0, 0, "wv")
        emit_vp_transposes(0, 0)
        emit_proj_unit(0, 0, "wq")
        emit_proj_unit(0, 1, "wv")
        emit_vp_transposes(0, 1)
        emit_proj_unit(0, 1, "wq")

        # ---- attention + interleaved fillers ----
        attn_of = {}
        pacc_of = {}
        z_of = {}

        def emit_attention_chunk(b, q):
            qs = slice(b * S + q * TCH, b * S + (q + 1) * TCH)
            paccA = ps_p.tile([128, TCH], f32, tag="pacc", name=f"pA_{b}_{q}")
            paccB = ps_p.tile([128, TCH], f32, tag="pacc", name=f"pB_{b}_{q}")
            zA = ps_z.tile([128, TCH], f32, tag="z", name=f"zA_{b}_{q}")
            zB = ps_z.tile([128, TCH], f32, tag="z", name=f"zB_{b}_{q}")
            pacc_of[(b, q)] = (paccA, paccB)
            z_of[(b, q)] = (zA, zB)
            sprev = None
            for kt in range(NKT + 1):
                if kt < NKT:
                    ks = slice(b * S + kt * 128, b * S + (kt + 1) * 128)
                    s = ps_s.tile([128, 2 * TCH], f32, tag="s",
                                  name=f"s_{b}_{q}_{kt}")
                    nc.tensor.matmul(s[:, 0:TCH], lhsT=KT[0:DK, ks],
                                     rhs=QT[0:DK, qs], start=True, stop=True)
                    nc.tensor.matmul(s[:, TCH:2 * TCH], lhsT=KT[DK:2 * DK, ks],
                                     rhs=QT[DK:2 * DK, qs],
                                     start=True, stop=True)
                    esb = esbp.tile([128, 2 * TCH], f16, tag="esb",
                                    name=f"esb_{b}_{q}_{kt}")
                    # one ACTIVATE per PSUM bank — a single FD=1024 read
                    # crossing the bank boundary is fatal on hardware
                    nc.scalar.activation(esb[:, 0:TCH], s[:, 0:TCH],
                                         Exp, scale=0.125)
                    nc.scalar.activation(esb[:, TCH:2 * TCH],
                                         s[:, TCH:2 * TCH],
                                         Exp, scale=0.125)
                if sprev is not None:
                    k0, esb0 = sprev
                    st, sp = (k0 == 0), (k0 == NKT - 1)
                    nc.tensor.matmul(paccA[0:DK, :], lhsT=vp[b][0][k0],
                                     rhs=esb0[:, 0:TCH], start=st, stop=sp)
                    if "noz" not in K_TEST:
                        nc.tensor.matmul(zA[64:65, :], lhsT=onecol,
                                         rhs=esb0[:, 0:TCH], start=st, stop=sp,
                                         tile_position=(0, 64))
                    nc.tensor.matmul(paccB[64:128, :], lhsT=vp[b][1][k0],
                                     rhs=esb0[:, TCH:2 * TCH], start=st,
                                     stop=sp, tile_position=(0, 64))
                    if "noz" not in K_TEST:
                        nc.tensor.matmul(zB[0:1, :], lhsT=onecol,
                                         rhs=esb0[:, TCH:2 * TCH], start=st,
                                         stop=sp, tile_position=(0, 0))
                if kt < NKT:
                    sprev = (kt, esb)

        def emit_normalize(b, q):
            """Reciprocal of the z rows, DRAM broadcast, and the two
            normalize-multiplies that assemble attn (f16) for (b, q)."""
            zA, zB = z_of[(b, q)]
            paccA, paccB = pacc_of[(b, q)]
            # gather both z rows into one SBUF tile so a single exact
            # reciprocal (FD-dominated cost) serves both heads;
            # reciprocal_approx_fast returns garbage on this runtime
            zg = rp.tile([128, TCH], f32, tag="zg", name=f"zg_{b}_{q}")
            nc.vector.tensor_copy(zg[64:65, :], zA[64:65, :])
            nc.vector.tensor_copy(zg[0:1, :], zB[0:1, :])
            rg = rp.tile([128, TCH], f32, tag="r", name=f"rg_{b}_{q}")
            nc.vector.reciprocal(rg[0:65, :], zg[0:65, :])
            scrA = drp.tile([1, TCH], f32, tag="scr", name=f"scrA_{b}_{q}")
            scrB = drp.tile([1, TCH], f32, tag="scr", name=f"scrB_{b}_{q}")
            nc.gpsimd.dma_start(scrA, rg[64:65, :])
            nc.gpsimd.dma_start(scrB, rg[0:1, :])
            rbA = rbp.tile([128, TCH], f32, tag="rb", name=f"rbA_{b}_{q}")
            rbB = rbp.tile([128, TCH], f32, tag="rb", name=f"rbB_{b}_{q}")
            nc.gpsimd.dma_start(
                rbA[0:DK, :],
                bass.AP(tensor=scrA.tensor, offset=scrA.offset,
                        ap=[[0, DK]] + scrA.ap[1:]))
            nc.gpsimd.dma_start(
                rbB[64:128, :],
                bass.AP(tensor=scrB.tensor, offset=scrB.offset,
                        ap=[[0, DK]] + scrB.ap[1:]))
            attn = attnp.tile([128, TCH], f16, tag="attn", name=f"at_{b}_{q}")
            nc.vector.tensor_mul(attn[0:DK, :], paccA[0:DK, :], rbA[0:DK, :])
            nc.vector.tensor_mul(attn[64:128, :], paccB[64:128, :],
                                 rbB[64:128, :])
            attn_of[(b, q)] = attn

        def emit_outproj(b, q):
            attn = attn_of[(b, q)]
            qs = slice(b * S + q * TCH, b * S + (q + 1) * TCH)
            for e in range(NE):
                po = ps_z.tile([128, TCH], f32, tag="z", name=f"po_{b}_{q}_{e}")
                nc.tensor.matmul(po, lhsT=wo_sb[e], rhs=attn,
                                 start=True, stop=True)
                ob = obp.tile([128, TCH], f16, tag="ob", name=f"ob_{b}_{q}_{e}")
                if e % 2 == 0:
                    nc.vector.tensor_copy(ob, po)
                else:
                    nc.scalar.copy(ob, po)
                nc.sync.dma_start(OT[e * 128:(e + 1) * 128, qs], ob)

        if "p1" in K_TEST:
            # phase-1-only crash bisection: dump QT/KT instead of attention
            for e in range(NE):
                obx = obp.tile([128, TCH], f16, tag="ob", name=f"obx_{e}")
                nc.vector.tensor_copy(obx, QT[:, e * TCH:(e + 1) * TCH])
                nc.sync.dma_start(OT[0:128, e * TCH:(e + 1) * TCH], obx)
        else:
            # batch-0 attention; batch-1 phase 1 rides the q-chunk boundaries
            emit_attention_chunk(0, 0)
            emit_normalize(0, 0)
            for t in range(NPC):
                emit_x_dma(1, t)
            for t in range(NPC):
                emit_proj_unit(1, t, "wv")
            for t in range(NPC):
                emit_vp_transposes(1, t)

            emit_attention_chunk(0, 1)
            emit_normalize(0, 1)
            emit_outproj(0, 0)
            for t in range(NPC):
                emit_proj_unit(1, t, "wk")

            emit_attention_chunk(0, 2)
            emit_normalize(0, 2)
            emit_outproj(0, 1)
            for t in range(NPC):
                emit_proj_unit(1, t, "wq")

            emit_attention_chunk(0, 3)
            emit_normalize(0, 3)
            emit_outproj(0, 2)

            emit_attention_chunk(1, 0)
            emit_normalize(1, 0)
            emit_outproj(0, 3)

            emit_attention_chunk(1, 1)
            emit_normalize(1, 1)
            emit_outproj(1, 0)

            emit_attention_chunk(1, 2)
            emit_normalize(1, 2)
            emit_outproj(1, 1)

            emit_attention_chunk(1, 3)
            emit_normalize(1, 3)
            emit_outproj(1, 2)
            emit_outproj(1, 3)

    nc.compile()
    return nc


def _get_program():
    if "nc" not in _CACHE:
        _CACHE["nc"] = _build_program()
    return _CACHE["nc"]


def _install_ntff_hook():
    """Provide the antenv.axon_hooks shim this container's antenv lacks so
    run_bass_kernel_spmd(trace=True) can capture NTFF profiles."""
    import sys
    import types

    try:
        import antenv

        if hasattr(antenv, "axon_hooks"):
            return
        mod = types.ModuleType("antenv.axon_hooks")
        mod._hook = None
        mod.set_axon_ntff_profile_hook = lambda h: setattr(mod, "_hook", h)
        mod.get_axon_ntff_profile_hook = lambda: mod._hook
        sys.modules["antenv.axon_hooks"] = mod
        antenv.axon_hooks = mod
        from trn_agent_boot.trn_boot import _ntff_profile_via_ctypes

        mod.set_axon_ntff_profile_hook(
            _ntff_profile_via_ctypes("/opt/axon/libaxon_pjrt.so"))
    except Exception:
        pass


def kernel(X, Wq, bq, Wk, bk, Wv, bv, Wo, bo, _profile=False, _trace_cores=None):
    from concourse.bass_utils import run_bass_kernel_spmd

    if _profile:
        _install_ntff_hook()

    nc = _get_program()

    XT = np.ascontiguousarray(
        np.asarray(X, np.float32).reshape(T, D).T).astype(np.float16)
    Wq, Wk, Wv, Wo = (np.asarray(w, np.float16) for w in (Wq, Wk, Wv, Wo))
    bq, bk, bv = (np.asarray(v, np.float32) for v in (bq, bk, bv))
    bo = np.asarray(bo, np.float32)

    in_maps = []
    for c in range(NCORES):
        cs = slice(c * COLS, (c + 1) * COLS)
        in_maps.append({
            "xt": XT,
            "wq": np.ascontiguousarray(Wq[:, cs]),
            "wk": np.ascontiguousarray(Wk[:, cs]),
            "wv": np.ascontiguousarray(Wv[:, cs]),
            "bq": np.ascontiguousarray(bq[cs].reshape(COLS, 1)),
            "bk": np.ascontiguousarray(bk[cs].reshape(COLS, 1)),
            "bv": np.ascontiguousarray(bv[cs].reshape(COLS, 1)),
            "wo": np.ascontiguousarray(Wo[cs, :]),
        })

    res = run_bass_kernel_spmd(
        nc, in_maps, core_ids=list(range(NCORES)),
        trace=_profile,
        trace_cores=(_trace_cores if _trace_cores is not None
                     else ([0] if _profile else None)),
    )

    ot = res.results[0]["ot"].astype(np.float32)
    for c in range(1, NCORES):
        ot += res.results[c]["ot"].astype(np.float32)
    out = (ot.T + bo).astype(np.float32).reshape(B, S, D)
    if _profile:
        kernel.last_exec_time_ns = res.exec_time_ns
        kernel.last_results = res
    return out


# revision 23
# speedup vs baseline: 1.2797x; 1.2797x over previous
"""Multi-head self-attention on 8 TRN2 NeuronCores.

Strategy: tensor-parallel over heads. Each core owns 2 of the 16 heads
(a 128-column slice of Wq/Wk/Wv and the matching 128-row slice of Wo) and
computes, for both batch elements, its QKV projection columns (transposed
[cols, tokens] layout), full attention for its (batch, head) pairs, and a
partial output projection. The host sums the 8 partial outputs (the
tensor-parallel all-reduce) and adds the output bias.

v2 changes vs the 441us baseline (which was PE-bound at 88% busy and ran
the whole attention phase HAM-throttled at 1.2 GHz):
  - every matmul is fp16 (the old f32r QKV/outproj lowered to
    fp32_mode=HIGH at 2 cycles/column),
  - V is transposed by the DMA xbar (dma_start_transpose) instead of the
    PE+DVE phase whose ~55% PE duty re-throttled the HAM clock gate,
  - scores for the 2 heads run concurrently on disjoint PE row groups,
  - the softmax denominator rides as a 1-column ones-matmul on a spare PE
    column group, concurrent with the AV matmul (tile_position col tiling),
  - exp runs as one FD=1024 ACTIVATE per key tile (both heads),
  - reciprocal_approx_fast instead of the 8-cycle/elem iterative divide,
  - batch-1 QKV projection and the output projections are emitted at
    query-chunk boundaries as dense PE filler that hides the softmax
    normalizer's DRAM broadcast roundtrip.
"""

import os

import numpy as np

# crash-bisection toggles (default = full kernel)
K_TEST = set(os.environ.get("K_TEST", "").split(","))

B, S, D, H, DK = 2, 2048, 1024, 16, 64
NCORES = 8
HPC = H // NCORES          # heads per core = 2
COLS = HPC * DK            # feature columns per core = 128
T = B * S                  # 4096 tokens
TCH = 512                  # query chunk (fp32 PSUM bank width)
PCH = 1024                 # phase-1 token chunk (f16 moving free dim)
ND = D // 128              # 8 contraction chunks
NE = D // 128              # 8 output-column chunks
NQ = S // TCH              # 4 query chunks per batch
NKT = S // 128             # 16 key tiles per batch
NPC = S // PCH             # 2 phase-1 chunks per batch

_CACHE = {}


def _build_program():
    from contextlib import ExitStack

    import concourse.bacc as bacc
    import concourse.mybir as mybir
    import concourse.tile as tile
    from concourse.masks import make_identity

    f32 = mybir.dt.float32
    f16 = mybir.dt.float16
    Exp = mybir.ActivationFunctionType.Exp

    nc = bacc.Bacc("TRN2", target_bir_lowering=False, debug=False,
                   num_devices=NCORES)

    XT = nc.dram_tensor("xt", [D, T], f16, kind="ExternalInput").ap()
    Wq = nc.dram_tensor("wq", [D, COLS], f16, kind="ExternalInput").ap()
    Wk = nc.dram_tensor("wk", [D, COLS], f16, kind="ExternalInput").ap()
    Wv = nc.dram_tensor("wv", [D, COLS], f16, kind="ExternalInput").ap()
    Bq = nc.dram_tensor("bq", [COLS, 1], f32, kind="ExternalInput").ap()
    Bk = nc.dram_tensor("bk", [COLS, 1], f32, kind="ExternalInput").ap()
    Bv = nc.dram_tensor("bv", [COLS, 1], f32, kind="ExternalInput").ap()
    Wo = nc.dram_tensor("wo", [COLS, D], f16, kind="ExternalInput").ap()
    OT = nc.dram_tensor("ot", [D, T], f16, kind="ExternalOutput").ap()

    import concourse.bass as bass

    with tile.TileContext(nc) as tc, ExitStack() as ctx:
        consts = ctx.enter_context(tc.tile_pool(name="consts", bufs=1))
        xtp = ctx.enter_context(tc.tile_pool(name="xtp", bufs=16))
        big = ctx.enter_context(tc.tile_pool(name="big", bufs=1))
        esbp = ctx.enter_context(tc.tile_pool(name="esbp", bufs=3))
        attnp = ctx.enter_context(tc.tile_pool(name="attnp", bufs=3))
        rp = ctx.enter_context(tc.tile_pool(name="rp", bufs=2))
        rbp = ctx.enter_context(tc.tile_pool(name="rbp", bufs=4))
        obp = ctx.enter_context(tc.tile_pool(name="obp", bufs=4))
        drp = ctx.enter_context(tc.tile_pool(name="drp", bufs=4, space="DRAM"))
        # PSUM: tag "s" [128,1024]x2 = 4 banks, tag "pacc" [128,512]x2 = 2,
        # tag "z" [128,512]x2 = 2 (shared by z accumulators and outproj).
        ps_s = ctx.enter_context(tc.tile_pool(name="ps_s", bufs=2, space="PSUM"))
        ps_p = ctx.enter_context(tc.tile_pool(name="ps_p", bufs=2, space="PSUM"))
        ps_z = ctx.enter_context(tc.tile_pool(name="ps_z", bufs=2, space="PSUM"))

        # ---- constants ----
        w_sb = {}
        b_sb = {}
        for nm, src, bsrc in (("wk", Wk, Bk), ("wv", Wv, Bv), ("wq", Wq, Bq)):
            w_sb[nm] = []
            for d in range(ND):
                wt = consts.tile([128, COLS], f16, name=f"{nm}_{d}")
                nc.sync.dma_start(wt, src[d * 128:(d + 1) * 128, :])
                w_sb[nm].append(wt)
            bt = consts.tile([COLS, 1], f32, name=f"b{nm}_sb")
            nc.sync.dma_start(bt, bsrc)
            b_sb[nm] = bt
        wo_sb = []
        for e in range(NE):
            wt = consts.tile([128, 128], f16, name=f"wo_{e}")
            nc.sync.dma_start(wt, Wo[:, e * 128:(e + 1) * 128])
            wo_sb.append(wt)
        onecol = consts.tile([128, 1], f16, name="onecol")
        nc.vector.memset(onecol, 1.0)
        ident = consts.tile([128, 128], f16, name="ident")
        make_identity(nc, ident)
        # pre-load the exp table set so the ~2.7us ACT_TABLE_LOAD happens
        # during phase 1 instead of at the first real exp
        dummy = consts.tile([128, 1], f16, name="dummy")
        nc.scalar.activation(dummy, onecol, Exp)

        QT = big.tile([COLS, T], f16, name="QT")
        KT = big.tile([COLS, T], f16, name="KT")
        proj_out = {"wq": QT, "wk": KT}
        # vpt[b][kt] = transposed V token tile [128 tokens, 128 features]
        # (head A = cols 0:64, head B = cols 64:128)
        vpt = [[None] * NKT for _ in range(B)]
        for b in range(B):
            for kt in range(NKT):
                vpt[b][kt] = consts.tile([128, 128], f16,
                                         name=f"vpt_{b}_{kt}")

        xts = {}

        def emit_x_dma(b, t):
            ts = slice(b * S + t * PCH, b * S + (t + 1) * PCH)
            cur = []
            for d in range(ND):
                xt_t = xtp.tile([128, PCH], f16, name=f"xt_{b}_{t}_{d}",
                                tag="xt")
                nc.gpsimd.dma_start(xt_t, XT[d * 128:(d + 1) * 128, ts])
                cur.append(xt_t)
            xts[(b, t)] = cur

        def emit_proj_unit(b, t, nm):
            """One phase-1 unit: [128, PCH] of one projection for batch b."""
            ts = slice(b * S + t * PCH, b * S + (t + 1) * PCH)
            acc = ps_s.tile([128, PCH], f32, tag="s", name=f"acc_{b}_{t}_{nm}")
            for half in range(2):
                hs = slice(half * TCH, (half + 1) * TCH)
                for d in range(ND):
                    nc.tensor.matmul(acc[:, hs], lhsT=w_sb[nm][d],
                                     rhs=xts[(b, t)][d][:, hs],
                                     start=(d == 0), stop=(d == ND - 1))
            if nm == "wv":
                vt = big.tile([COLS, PCH], f16, tag="vt", bufs=2,
                              name=f"vt_{b}_{t}")
                nc.vector.tensor_scalar_add(vt, acc, b_sb[nm])
                vts[(b, t)] = vt
            else:
                nc.vector.tensor_scalar_add(proj_out[nm][:, ts], acc, b_sb[nm])

        vts = {}

        def emit_vp_transposes(b, t):
            """PE transposes of V token tiles for batch b, chunk t."""
            vt = vts[(b, t)]
            for kt in range(t * PCH // 128, (t + 1) * PCH // 128):
                c0 = (kt * 128) % PCH
                ptrz = ps_z.tile([128, TCH], f32, tag="z",
                                 name=f"ptr_{b}_{kt}")
                ptr = ptrz.bitcast(f16)[:, 0:128]
                nc.tensor.transpose(ptr, vt[:, c0:c0 + 128], ident)
                nc.vector.tensor_copy(vpt[b][kt], ptr)

        # ---- phase 1 for batch 0 ----
        # K first (scores need the full key range), then V chunk 0 (vp tiles
        # for the first AV matmuls), then Q chunk 0 (first scores) — gets the
        # scalar engine streaming exps as early as possible.
        for t in range(NPC):
            emit_x_dma(0, t)
        emit_proj_unit(0, 0, "wk")
        emit_proj_unit(0, 1, "wk")
        emit_proj_unit(0, 0, "wv")
        emit_vp_transposes(0, 0)
        emit_proj_unit(0, 0, "wq")
        emit_proj_unit(0, 1, "wv")
        emit_vp_transposes(0, 1)
        emit_proj_unit(0, 1, "wq")
        # prefetch batch-1 activations while batch-0 attention runs
        for t in range(NPC):
            emit_x_dma(1, t)

        # ---- attention + interleaved fillers ----
        attn_of = {}
        pacc_of = {}
        z_of = {}

        def emit_attention_chunk(b, q):
            qs = slice(b * S + q * TCH, b * S + (q + 1) * TCH)
            paccA = ps_p.tile([128, TCH], f32, tag="pacc", name=f"pA_{b}_{q}")
            paccB = ps_p.tile([128, TCH], f32, tag="pacc", name=f"pB_{b}_{q}")
            zA = ps_z.tile([128, TCH], f32, tag="z", name=f"zA_{b}_{q}")
            zB = ps_z.tile([128, TCH], f32, tag="z", name=f"zB_{b}_{q}")
            pacc_of[(b, q)] = (paccA, paccB)
            z_of[(b, q)] = (zA, zB)
            sprev = None
            for kt in range(NKT + 1):
                if kt < NKT:
                    ks = slice(b * S + kt * 128, b * S + (kt + 1) * 128)
                    s = ps_s.tile([128, 2 * TCH], f32, tag="s",
                                  name=f"s_{b}_{q}_{kt}")
                    nc.tensor.matmul(s[:, 0:TCH], lhsT=KT[0:DK, ks],
                                     rhs=QT[0:DK, qs], start=True, stop=True)
                    nc.tensor.matmul(s[:, TCH:2 * TCH], lhsT=KT[DK:2 * DK, ks],
                                     rhs=QT[DK:2 * DK, qs],
                                     start=True, stop=True)
                    esb = esbp.tile([128, 2 * TCH], f16, tag="esb",
                                    name=f"esb_{b}_{q}_{kt}")
                    # one ACTIVATE per PSUM bank — a single FD=1024 read
                    # crossing the bank boundary is fatal on hardware
                    nc.scalar.activation(esb[:, 0:TCH], s[:, 0:TCH],
                                         Exp, scale=0.125)
                    nc.scalar.activation(esb[:, TCH:2 * TCH],
                                         s[:, TCH:2 * TCH],
                                         Exp, scale=0.125)
                if sprev is not None:
                    k0, esb0 = sprev
                    st, sp = (k0 == 0), (k0 == NKT - 1)
                    nc.tensor.matmul(paccA[0:DK, :], lhsT=vpt[b][k0][:, 0:DK],
                                     rhs=esb0[:, 0:TCH], start=st, stop=sp)
                    nc.tensor.matmul(zA[64:65, :], lhsT=onecol,
                                     rhs=esb0[:, 0:TCH], start=st, stop=sp,
                                     tile_position=(0, 64))
                    nc.tensor.matmul(paccB[64:128, :],
                                     lhsT=vpt[b][k0][:, DK:2 * DK],
                                     rhs=esb0[:, TCH:2 * TCH], start=st,
                                     stop=sp, tile_position=(0, 64))
                    nc.tensor.matmul(zB[0:1, :], lhsT=onecol,
                                     rhs=esb0[:, TCH:2 * TCH], start=st,
                                     stop=sp, tile_position=(0, 0))
                if kt < NKT:
                    sprev = (kt, esb)

        def emit_normalize(b, q):
            """Reciprocal of the z rows, DRAM broadcast, and the two
            normalize-multiplies that assemble attn (f16) for (b, q)."""
            zA, zB = z_of[(b, q)]
            paccA, paccB = pacc_of[(b, q)]
            # gather both z rows into one SBUF tile so a single exact
            # reciprocal (FD-dominated cost) serves both heads;
            # reciprocal_approx_fast returns garbage on this runtime
            zg = rp.tile([128, TCH], f32, tag="zg", name=f"zg_{b}_{q}")
            nc.vector.memset(zg[0:64, :], 1.0)
            nc.vector.tensor_copy(zg[64:65, :], zA[64:65, :])
            nc.vector.tensor_copy(zg[0:1, :], zB[0:1, :])
            rg = rp.tile([128, TCH], f32, tag="r", name=f"rg_{b}_{q}")
            nc.vector.reciprocal(rg[0:65, :], zg[0:65, :])
            scrA = drp.tile([1, TCH], f32, tag="scr", name=f"scrA_{b}_{q}")
            scrB = drp.tile([1, TCH], f32, tag="scr", name=f"scrB_{b}_{q}")
            nc.gpsimd.dma_start(scrA, rg[64:65, :])
            nc.gpsimd.dma_start(scrB, rg[0:1, :])
            rbA = rbp.tile([128, TCH], f32, tag="rb", name=f"rbA_{b}_{q}")
            rbB = rbp.tile([128, TCH], f32, tag="rb", name=f"rbB_{b}_{q}")
            nc.gpsimd.dma_start(
                rbA[0:DK, :],
                bass.AP(tensor=scrA.tensor, offset=scrA.offset,
                        ap=[[0, DK]] + scrA.ap[1:]))
            nc.gpsimd.dma_start(
                rbB[64:128, :],
                bass.AP(tensor=scrB.tensor, offset=scrB.offset,
                        ap=[[0, DK]] + scrB.ap[1:]))
            attn = attnp.tile([128, TCH], f16, tag="attn", name=f"at_{b}_{q}")
            nc.vector.tensor_mul(attn[0:DK, :], paccA[0:DK, :], rbA[0:DK, :])
            nc.vector.tensor_mul(attn[64:128, :], paccB[64:128, :],
                                 rbB[64:128, :])
            attn_of[(b, q)] = attn

        def emit_outproj(b, q):
            attn = attn_of[(b, q)]
            qs = slice(b * S + q * TCH, b * S + (q + 1) * TCH)
            for e in range(NE):
                po = ps_z.tile([128, TCH], f32, tag="z", name=f"po_{b}_{q}_{e}")
                nc.tensor.matmul(po, lhsT=wo_sb[e], rhs=attn,
                                 start=True, stop=True)
                ob = obp.tile([128, TCH], f16, tag="ob", name=f"ob_{b}_{q}_{e}")
                nc.vector.tensor_copy(ob, po)
                nc.sync.dma_start(OT[e * 128:(e + 1) * 128, qs], ob)

        if "p1" in K_TEST:
            # phase-1-only crash bisection: dump QT/KT instead of attention
            for e in range(NE):
                obx = obp.tile([128, TCH], f16, tag="ob", name=f"obx_{e}")
                nc.vector.tensor_copy(obx, QT[:, e * TCH:(e + 1) * TCH])
                nc.sync.dma_start(OT[0:128, e * TCH:(e + 1) * TCH], obx)
        else:
            # batch-0 attention; batch-1 phase 1 rides the q-chunk boundaries
            emit_attention_chunk(0, 0)
            emit_normalize(0, 0)
            for t in range(NPC):
                emit_proj_unit(1, t, "wv")
                emit_vp_transposes(1, t)

            emit_attention_chunk(0, 1)
            emit_normalize(0, 1)
            emit_outproj(0, 0)
            for t in range(NPC):
                emit_proj_unit(1, t, "wk")

            emit_attention_chunk(0, 2)
            emit_normalize(0, 2)
            emit_outproj(0, 1)
            for t in range(NPC):
                emit_proj_unit(1, t, "wq")

            emit_attention_chunk(0, 3)
            emit_normalize(0, 3)
            emit_outproj(0, 2)

            emit_attention_chunk(1, 0)
            emit_normalize(1, 0)
            emit_outproj(0, 3)

            emit_attention_chunk(1, 1)
            emit_normalize(1, 1)
            emit_outproj(1, 0)

            emit_attention_chunk(1, 2)
            emit_normalize(1, 2)
            emit_outproj(1, 1)

            emit_attention_chunk(1, 3)
            emit_normalize(1, 3)
            emit_outproj(1, 2)
            emit_outproj(1, 3)

    nc.compile()
    return nc


def _get_program():
    if "nc" not in _CACHE:
        _CACHE["nc"] = _build_program()
    return _CACHE["nc"]


def _install_ntff_hook():
    """Provide the antenv.axon_hooks shim this container's antenv lacks so
    run_bass_kernel_spmd(trace=True) can capture NTFF profiles."""
    import sys
    import types

    try:
        import antenv

        if hasattr(antenv, "axon_hooks"):
            return
        mod = types.ModuleType("antenv.axon_hooks")
        mod._hook = None
        mod.set_axon_ntff_profile_hook = lambda h: setattr(mod, "_hook", h)
        mod.get_axon_ntff_profile_hook = lambda: mod._hook
        sys.modules["antenv.axon_hooks"] = mod
        antenv.axon_hooks = mod
        from trn_agent_boot.trn_boot import _ntff_profile_via_ctypes

        mod.set_axon_ntff_profile_hook(
            _ntff_profile_via_ctypes("/opt/axon/libaxon_pjrt.so"))
    except Exception:
        pass


def kernel(X, Wq, bq, Wk, bk, Wv, bv, Wo, bo, _profile=False, _trace_cores=None):
    from concourse.bass_utils import run_bass_kernel_spmd

    if _profile:
        _install_ntff_hook()

    nc = _get_program()

    XT = np.ascontiguousarray(
        np.asarray(X, np.float32).reshape(T, D).T).astype(np.float16)
    Wq, Wk, Wv, Wo = (np.asarray(w, np.float16) for w in (Wq, Wk, Wv, Wo))
    bq, bk, bv = (np.asarray(v, np.float32) for v in (bq, bk, bv))
    bo = np.asarray(bo, np.float32)

    in_maps = []
    for c in range(NCORES):
        cs = slice(c * COLS, (c + 1) * COLS)
        in_maps.append({
            "xt": XT,
            "wq": np.ascontiguousarray(Wq[:, cs]),
            "wk": np.ascontiguousarray(Wk[:, cs]),
            "wv": np.ascontiguousarray(Wv[:, cs]),
            "bq": np.ascontiguousarray(bq[cs].reshape(COLS, 1)),
            "bk": np.ascontiguousarray(bk[cs].reshape(COLS, 1)),
            "bv": np.ascontiguousarray(bv[cs].reshape(COLS, 1)),
            "wo": np.ascontiguousarray(Wo[cs, :]),
        })

    res = run_bass_kernel_spmd(
        nc, in_maps, core_ids=list(range(NCORES)),
        trace=_profile,
        trace_cores=(_trace_cores if _trace_cores is not None
                     else ([0] if _profile else None)),
    )

    ot = res.results[0]["ot"].astype(np.float32)
    for c in range(1, NCORES):
        ot += res.results[c]["ot"].astype(np.float32)
    out = (ot.T + bo).astype(np.float32).reshape(B, S, D)
    if _profile:
        kernel.last_exec_time_ns = res.exec_time_ns
        kernel.last_results = res
    return out


# revision 27
# speedup vs baseline: 1.3757x; 1.0750x over previous
"""Multi-head self-attention on 8 TRN2 NeuronCores.

Strategy: tensor-parallel over heads. Each core owns 2 of the 16 heads
(a 128-column slice of Wq/Wk/Wv and the matching 128-row slice of Wo) and
computes, for both batch elements, its QKV projection columns (transposed
[cols, tokens] layout), full attention for its (batch, head) pairs, and a
partial output projection. The host sums the 8 partial outputs (the
tensor-parallel all-reduce) and adds the output bias.

v2 changes vs the 441us baseline (which was PE-bound at 88% busy and ran
the whole attention phase HAM-throttled at 1.2 GHz):
  - every matmul is fp16 (the old f32r QKV/outproj lowered to
    fp32_mode=HIGH at 2 cycles/column),
  - V is transposed by the DMA xbar (dma_start_transpose) instead of the
    PE+DVE phase whose ~55% PE duty re-throttled the HAM clock gate,
  - scores for the 2 heads run concurrently on disjoint PE row groups,
  - the softmax denominator rides as a 1-column ones-matmul on a spare PE
    column group, concurrent with the AV matmul (tile_position col tiling),
  - exp runs as one FD=1024 ACTIVATE per key tile (both heads),
  - reciprocal_approx_fast instead of the 8-cycle/elem iterative divide,
  - batch-1 QKV projection and the output projections are emitted at
    query-chunk boundaries as dense PE filler that hides the softmax
    normalizer's DRAM broadcast roundtrip.
"""

import os

import numpy as np

# crash-bisection toggles (default = full kernel)
K_TEST = set(os.environ.get("K_TEST", "").split(","))

B, S, D, H, DK = 2, 2048, 1024, 16, 64
NCORES = 8
HPC = H // NCORES          # heads per core = 2
COLS = HPC * DK            # feature columns per core = 128
T = B * S                  # 4096 tokens
TCH = 512                  # query chunk (fp32 PSUM bank width)
PCH = 1024                 # phase-1 token chunk (f16 moving free dim)
ND = D // 128              # 8 contraction chunks
NE = D // 128              # 8 output-column chunks
NQ = S // TCH              # 4 query chunks per batch
NKT = S // 128             # 16 key tiles per batch
NPC = S // PCH             # 2 phase-1 chunks per batch

_CACHE = {}


def _build_program():
    from contextlib import ExitStack

    import concourse.bacc as bacc
    import concourse.mybir as mybir
    import concourse.tile as tile
    from concourse.masks import make_identity

    f32 = mybir.dt.float32
    f16 = mybir.dt.float16
    Exp = mybir.ActivationFunctionType.Exp

    nc = bacc.Bacc("TRN2", target_bir_lowering=False, debug=False,
                   num_devices=NCORES)

    XT = nc.dram_tensor("xt", [D, T], f16, kind="ExternalInput").ap()
    Wq = nc.dram_tensor("wq", [D, COLS], f16, kind="ExternalInput").ap()
    Wk = nc.dram_tensor("wk", [D, COLS], f16, kind="ExternalInput").ap()
    Wv = nc.dram_tensor("wv", [D, COLS], f16, kind="ExternalInput").ap()
    Bq = nc.dram_tensor("bq", [COLS, 1], f32, kind="ExternalInput").ap()
    Bk = nc.dram_tensor("bk", [COLS, 1], f32, kind="ExternalInput").ap()
    Bv = nc.dram_tensor("bv", [COLS, 1], f32, kind="ExternalInput").ap()
    Wo = nc.dram_tensor("wo", [COLS, D], f16, kind="ExternalInput").ap()
    OT = nc.dram_tensor("ot", [D, T], f16, kind="ExternalOutput").ap()

    import concourse.bass as bass

    with tile.TileContext(nc) as tc, ExitStack() as ctx:
        consts = ctx.enter_context(tc.tile_pool(name="consts", bufs=1))
        xtp = ctx.enter_context(tc.tile_pool(name="xtp", bufs=16))
        big = ctx.enter_context(tc.tile_pool(name="big", bufs=1))
        esbp = ctx.enter_context(tc.tile_pool(name="esbp", bufs=3))
        attnp = ctx.enter_context(tc.tile_pool(name="attnp", bufs=3))
        arp = ctx.enter_context(tc.tile_pool(name="arp", bufs=3))
        rp = ctx.enter_context(tc.tile_pool(name="rp", bufs=2))
        rbp = ctx.enter_context(tc.tile_pool(name="rbp", bufs=4))
        obp = ctx.enter_context(tc.tile_pool(name="obp", bufs=4))
        drp = ctx.enter_context(tc.tile_pool(name="drp", bufs=4, space="DRAM"))
        # PSUM (8 banks): "s" [128,1024]x2 = 4, "pacc" x2 = 2, "z" x1, "po" x1
        ps_s = ctx.enter_context(tc.tile_pool(name="ps_s", bufs=2, space="PSUM"))
        ps_p = ctx.enter_context(tc.tile_pool(name="ps_p", bufs=2, space="PSUM"))
        ps_z = ctx.enter_context(tc.tile_pool(name="ps_z", bufs=1, space="PSUM"))
        ps_o = ctx.enter_context(tc.tile_pool(name="ps_o", bufs=1, space="PSUM"))

        # ---- constants ----
        w_sb = {}
        b_sb = {}
        for nm, src, bsrc in (("wk", Wk, Bk), ("wv", Wv, Bv), ("wq", Wq, Bq)):
            w_sb[nm] = []
            for d in range(ND):
                wt = consts.tile([128, COLS], f16, name=f"{nm}_{d}")
                nc.sync.dma_start(wt, src[d * 128:(d + 1) * 128, :])
                w_sb[nm].append(wt)
            bt = consts.tile([COLS, 1], f32, name=f"b{nm}_sb")
            nc.sync.dma_start(bt, bsrc)
            b_sb[nm] = bt
        wo_sb = []
        for e in range(NE):
            wt = consts.tile([128, 128], f16, name=f"wo_{e}")
            nc.sync.dma_start(wt, Wo[:, e * 128:(e + 1) * 128])
            wo_sb.append(wt)
        onecol = consts.tile([128, 1], f16, name="onecol")
        nc.vector.memset(onecol, 1.0)
        ident = consts.tile([128, 128], f16, name="ident")
        make_identity(nc, ident)
        # pre-load the exp table set so the ~2.7us ACT_TABLE_LOAD happens
        # during phase 1 instead of at the first real exp
        dummy = consts.tile([128, 1], f16, name="dummy")
        nc.scalar.activation(dummy, onecol, Exp)

        QT = big.tile([COLS, T], f16, name="QT")
        KT = big.tile([COLS, T], f16, name="KT")
        proj_out = {"wq": QT, "wk": KT}
        # vpt[b][kt] = transposed V token tile [128 tokens, 128 features]
        # (head A = cols 0:64, head B = cols 64:128)
        vpt = [[None] * NKT for _ in range(B)]
        for b in range(B):
            for kt in range(NKT):
                vpt[b][kt] = consts.tile([128, 128], f16,
                                         name=f"vpt_{b}_{kt}")

        xts = {}

        def emit_x_dma(b, t):
            ts = slice(b * S + t * PCH, b * S + (t + 1) * PCH)
            cur = []
            for d in range(ND):
                xt_t = xtp.tile([128, PCH], f16, name=f"xt_{b}_{t}_{d}",
                                tag="xt")
                nc.gpsimd.dma_start(xt_t, XT[d * 128:(d + 1) * 128, ts])
                cur.append(xt_t)
            xts[(b, t)] = cur

        def emit_proj_unit(b, t, nm):
            """One phase-1 unit: [128, PCH] of one projection for batch b."""
            ts = slice(b * S + t * PCH, b * S + (t + 1) * PCH)
            acc = ps_s.tile([128, PCH], f32, tag="s", name=f"acc_{b}_{t}_{nm}")
            for half in range(2):
                hs = slice(half * TCH, (half + 1) * TCH)
                for d in range(ND):
                    nc.tensor.matmul(acc[:, hs], lhsT=w_sb[nm][d],
                                     rhs=xts[(b, t)][d][:, hs],
                                     start=(d == 0), stop=(d == ND - 1))
            if nm == "wv":
                vt = big.tile([COLS, PCH], f16, tag="vt", bufs=2,
                              name=f"vt_{b}_{t}")
                nc.vector.tensor_scalar_add(vt, acc, b_sb[nm])
                vts[(b, t)] = vt
            else:
                nc.vector.tensor_scalar_add(proj_out[nm][:, ts], acc, b_sb[nm])

        vts = {}

        # ---- emission ----
        # PSUM budget (8 banks): tag "s" [128,1024]x2 = 4 (scores pairs +
        # phase-1 accumulators), tag "pacc" [128,512]x2 = 2 (combined AV
        # accumulator per q-chunk, double-buffered), tag "z" x1 = 1 (both
        # softmax-denominator rows), tag "po" x1 = 1 (outproj + V-transposes).
        #
        # Head A's kt0 matmul carries start=True and clears the whole bank;
        # head B's kt0 matmul uses start=False and lands on cleared
        # has_written bits, so it overwrites rather than accumulates stale
        # data. Emission order (A before B at kt0) is load-bearing.
        attn_of = {}
        araw_of = {}
        pacc_of = {}
        z_of = {}
        rb_of = {}

        def emit_attention_chunk(b, q, fillers=()):
            qs = slice(b * S + q * TCH, b * S + (q + 1) * TCH)
            pacc = ps_p.tile([128, TCH], f32, tag="pacc", name=f"p_{b}_{q}")
            zt = ps_z.tile([128, TCH], f32, tag="z", name=f"z_{b}_{q}")
            pacc_of[(b, q)] = pacc
            z_of[(b, q)] = zt
            fillers = list(fillers)
            fi = 0
            sprev = None
            for kt in range(NKT + 1):
                if kt < NKT:
                    ks = slice(b * S + kt * 128, b * S + (kt + 1) * 128)
                    s = ps_s.tile([128, 2 * TCH], f32, tag="s",
                                  name=f"s_{b}_{q}_{kt}")
                    nc.tensor.matmul(s[:, 0:TCH], lhsT=KT[0:DK, ks],
                                     rhs=QT[0:DK, qs], start=True, stop=True)
                    nc.tensor.matmul(s[:, TCH:2 * TCH], lhsT=KT[DK:2 * DK, ks],
                                     rhs=QT[DK:2 * DK, qs],
                                     start=True, stop=True)
                    esb = esbp.tile([128, 2 * TCH], f16, tag="esb",
                                    name=f"esb_{b}_{q}_{kt}")
                    # one ACTIVATE per PSUM bank — a single FD=1024 read
                    # crossing the bank boundary is fatal on hardware
                    nc.scalar.activation(esb[:, 0:TCH], s[:, 0:TCH],
                                         Exp, scale=0.125)
                    nc.scalar.activation(esb[:, TCH:2 * TCH],
                                         s[:, TCH:2 * TCH],
                                         Exp, scale=0.125)
                if sprev is not None:
                    k0, esb0 = sprev
                    st, sp = (k0 == 0), (k0 == NKT - 1)
                    nc.tensor.matmul(pacc[0:DK, :], lhsT=vpt[b][k0][:, 0:DK],
                                     rhs=esb0[:, 0:TCH], start=st, stop=sp)
                    nc.tensor.matmul(zt[64:65, :], lhsT=onecol,
                                     rhs=esb0[:, 0:TCH], start=st, stop=sp,
                                     tile_position=(0, 64))
                    nc.tensor.matmul(pacc[64:128, :],
                                     lhsT=vpt[b][k0][:, DK:2 * DK],
                                     rhs=esb0[:, TCH:2 * TCH],
                                     start=st, stop=sp,
                                     tile_position=(0, 64),
                                     skip_group_check=True)
                    nc.tensor.matmul(zt[0:1, :], lhsT=onecol,
                                     rhs=esb0[:, TCH:2 * TCH],
                                     start=st, stop=sp,
                                     tile_position=(0, 0),
                                     skip_group_check=True)
                    if k0 >= 1 and fi < len(fillers):
                        fillers[fi]()
                        fi += 1
                if kt < NKT:
                    sprev = (kt, esb)
            while fi < len(fillers):
                fillers[fi]()
                fi += 1

        def emit_muls(bq):
            """Normalize staged attention with the broadcast reciprocals."""
            ar = araw_of[bq]
            rbA, rbB = rb_of[bq]
            attn = attnp.tile([128, TCH], f16, tag="attn",
                              name=f"at_{bq[0]}_{bq[1]}")
            nc.vector.tensor_mul(attn[0:DK, :], ar[0:DK, :], rbA[0:DK, :])
            nc.vector.tensor_mul(attn[64:128, :], ar[64:128, :],
                                 rbB[64:128, :])
            attn_of[bq] = attn

        def emit_epilogue(b, q, prev=None):
            """Stage pacc/z out of PSUM (freeing the banks fast), normalize
            the previous chunk, and launch this chunk's reciprocal +
            DRAM-broadcast roundtrip."""
            pacc = pacc_of[(b, q)]
            zt = z_of[(b, q)]
            ar = arp.tile([128, TCH], f16, tag="araw", name=f"ar_{b}_{q}")
            nc.vector.tensor_copy(ar[0:DK, :], pacc[0:DK, :])
            nc.vector.tensor_copy(ar[64:128, :], pacc[64:128, :])
            araw_of[(b, q)] = ar
            zg = rp.tile([128, TCH], f32, tag="zg", name=f"zg_{b}_{q}")
            nc.vector.memset(zg[0:64, :], 1.0)
            nc.vector.tensor_copy(zg[64:65, :], zt[64:65, :])
            nc.vector.tensor_copy(zg[0:1, :], zt[0:1, :])
            if prev is not None:
                emit_muls(prev)
            rg = rp.tile([128, TCH], f32, tag="r", name=f"rg_{b}_{q}")
            nc.vector.reciprocal(rg[0:65, :], zg[0:65, :])
            scrA = drp.tile([1, TCH], f32, tag="scr", name=f"scrA_{b}_{q}")
            scrB = drp.tile([1, TCH], f32, tag="scr", name=f"scrB_{b}_{q}")
            nc.gpsimd.dma_start(scrA, rg[64:65, :])
            nc.gpsimd.dma_start(scrB, rg[0:1, :])
            rbA = rbp.tile([128, TCH], f32, tag="rb", name=f"rbA_{b}_{q}")
            rbB = rbp.tile([128, TCH], f32, tag="rb", name=f"rbB_{b}_{q}")
            nc.gpsimd.dma_start(
                rbA[0:DK, :],
                bass.AP(tensor=scrA.tensor, offset=scrA.offset,
                        ap=[[0, DK]] + scrA.ap[1:]))
            nc.gpsimd.dma_start(
                rbB[64:128, :],
                bass.AP(tensor=scrB.tensor, offset=scrB.offset,
                        ap=[[0, DK]] + scrB.ap[1:]))
            rb_of[(b, q)] = (rbA, rbB)

        def mk_po(b, q, e):
            def f():
                attn = attn_of[(b, q)]
                qs = slice(b * S + q * TCH, b * S + (q + 1) * TCH)
                po = ps_o.tile([128, TCH], f32, tag="po",
                               name=f"po_{b}_{q}_{e}")
                nc.tensor.matmul(po, lhsT=wo_sb[e], rhs=attn,
                                 start=True, stop=True)
                ob = obp.tile([128, TCH], f16, tag="ob",
                              name=f"ob_{b}_{q}_{e}")
                nc.vector.tensor_copy(ob, po)
                nc.sync.dma_start(OT[e * 128:(e + 1) * 128, qs], ob)
            return f

        def po_fillers(b, q):
            return [mk_po(b, q, e) for e in range(NE)]

        def mk_proj(b, t, nm):
            return lambda: emit_proj_unit(b, t, nm)

        def mk_tp(b, ktiles):
            def f():
                for kt in ktiles:
                    t = (kt * 128) // PCH
                    c0 = (kt * 128) % PCH
                    ptrz = ps_o.tile([128, TCH], f32, tag="po",
                                     name=f"ptr_{b}_{kt}")
                    ptr = ptrz.bitcast(f16)[:, 0:128]
                    nc.tensor.transpose(ptr, vts[(b, t)][:, c0:c0 + 128],
                                        ident)
                    nc.vector.tensor_copy(vpt[b][kt], ptr)
            return f

        def mk_xdma(b):
            def f():
                for t in range(NPC):
                    emit_x_dma(b, t)
            return f

        # phase-1 head: only what batch-0 q-chunk 0 needs up front; the rest
        # streams in as kt-loop fillers so the PE never idles and the scalar
        # engine starts exping as early as possible
        for t in range(NPC):
            emit_x_dma(0, t)
        emit_proj_unit(0, 0, "wk")
        emit_proj_unit(0, 0, "wq")
        emit_proj_unit(0, 0, "wv")
        mk_tp(0, range(0, 8))()

        emit_attention_chunk(0, 0, [
            mk_proj(0, 1, "wk"), mk_proj(0, 1, "wv"),
            mk_tp(0, [8, 9]), mk_tp(0, [10, 11]),
            mk_tp(0, [12, 13]), mk_tp(0, [14, 15]),
        ])
        emit_epilogue(0, 0)

        emit_attention_chunk(0, 1, [
            mk_proj(0, 1, "wq"), mk_xdma(1),
            mk_proj(1, 0, "wv"),
        ])
        emit_epilogue(0, 1, prev=(0, 0))

        emit_attention_chunk(0, 2, po_fillers(0, 0) + [
            mk_proj(1, 1, "wv"),
            mk_tp(1, [0, 1]), mk_tp(1, [2, 3]),
        ])
        emit_epilogue(0, 2, prev=(0, 1))

        emit_attention_chunk(0, 3, po_fillers(0, 1) + [
            mk_tp(1, [4, 5]), mk_tp(1, [6, 7]),
            mk_proj(1, 0, "wk"), mk_proj(1, 1, "wk"),
            mk_proj(1, 0, "wq"),
        ])
        emit_epilogue(0, 3, prev=(0, 2))

        emit_attention_chunk(1, 0, [
            mk_tp(1, [8, 9]), mk_tp(1, [10, 11]),
            mk_tp(1, [12, 13]), mk_tp(1, [14, 15]),
        ] + po_fillers(0, 2))
        emit_epilogue(1, 0, prev=(0, 3))

        emit_attention_chunk(1, 1, [mk_proj(1, 1, "wq")]
                             + po_fillers(0, 3))
        emit_epilogue(1, 1, prev=(1, 0))

        emit_attention_chunk(1, 2, po_fillers(1, 0))
        emit_epilogue(1, 2, prev=(1, 1))

        emit_attention_chunk(1, 3, po_fillers(1, 1))
        emit_epilogue(1, 3, prev=(1, 2))

        for f in po_fillers(1, 2):
            f()
        emit_muls((1, 3))
        for f in po_fillers(1, 3):
            f()

    nc.compile()
    return nc


def _get_program():
    if "nc" not in _CACHE:
        _CACHE["nc"] = _build_program()
    return _CACHE["nc"]


def _install_ntff_hook():
    """Provide the antenv.axon_hooks shim this container's antenv lacks so
    run_bass_kernel_spmd(trace=True) can capture NTFF profiles."""
    import sys
    import types

    try:
        import antenv

        if hasattr(antenv, "axon_hooks"):
            return
        mod = types.ModuleType("antenv.axon_hooks")
        mod._hook = None
        mod.set_axon_ntff_profile_hook = lambda h: setattr(mod, "_hook", h)
        mod.get_axon_ntff_profile_hook = lambda: mod._hook
        sys.modules["antenv.axon_hooks"] = mod
        antenv.axon_hooks = mod
        from trn_agent_boot.trn_boot import _ntff_profile_via_ctypes

        mod.set_axon_ntff_profile_hook(
            _ntff_profile_via_ctypes("/opt/axon/libaxon_pjrt.so"))
    except Exception:
        pass


def kernel(X, Wq, bq, Wk, bk, Wv, bv, Wo, bo, _profile=False, _trace_cores=None):
    from concourse.bass_utils import run_bass_kernel_spmd

    if _profile:
        _install_ntff_hook()

    nc = _get_program()

    XT = np.ascontiguousarray(
        np.asarray(X, np.float32).reshape(T, D).T).astype(np.float16)
    Wq, Wk, Wv, Wo = (np.asarray(w, np.float16) for w in (Wq, Wk, Wv, Wo))
    bq, bk, bv = (np.asarray(v, np.float32) for v in (bq, bk, bv))
    bo = np.asarray(bo, np.float32)

    in_maps = []
    for c in range(NCORES):
        cs = slice(c * COLS, (c + 1) * COLS)
        in_maps.append({
            "xt": XT,
            "wq": np.ascontiguousarray(Wq[:, cs]),
            "wk": np.ascontiguousarray(Wk[:, cs]),
            "wv": np.ascontiguousarray(Wv[:, cs]),
            "bq": np.ascontiguousarray(bq[cs].reshape(COLS, 1)),
            "bk": np.ascontiguousarray(bk[cs].reshape(COLS, 1)),
            "bv": np.ascontiguousarray(bv[cs].reshape(COLS, 1)),
            "wo": np.ascontiguousarray(Wo[cs, :]),
        })

    res = run_bass_kernel_spmd(
        nc, in_maps, core_ids=list(range(NCORES)),
        trace=_profile,
        trace_cores=(_trace_cores if _trace_cores is not None
                     else ([0] if _profile else None)),
    )

    ot = res.results[0]["ot"].astype(np.float32)
    for c in range(1, NCORES):
        ot += res.results[c]["ot"].astype(np.float32)
    out = (ot.T + bo).astype(np.float32).reshape(B, S, D)
    if _profile:
        kernel.last_exec_time_ns = res.exec_time_ns
        kernel.last_results = res
    return out


# revision 29
# speedup vs baseline: 1.4790x; 1.0752x over previous
"""Multi-head self-attention on 8 TRN2 NeuronCores.

Strategy: tensor-parallel over heads. Each core owns 2 of the 16 heads
(a 128-column slice of Wq/Wk/Wv and the matching 128-row slice of Wo) and
computes, for both batch elements, its QKV projection columns (transposed
[cols, tokens] layout), full attention for its (batch, head) pairs, and a
partial output projection. The host sums the 8 partial outputs (the
tensor-parallel all-reduce) and adds the output bias.

v2 changes vs the 441us baseline (which was PE-bound at 88% busy and ran
the whole attention phase HAM-throttled at 1.2 GHz):
  - every matmul is fp16 (the old f32r QKV/outproj lowered to
    fp32_mode=HIGH at 2 cycles/column),
  - V is transposed by the DMA xbar (dma_start_transpose) instead of the
    PE+DVE phase whose ~55% PE duty re-throttled the HAM clock gate,
  - scores for the 2 heads run concurrently on disjoint PE row groups,
  - the softmax denominator rides as a 1-column ones-matmul on a spare PE
    column group, concurrent with the AV matmul (tile_position col tiling),
  - exp runs as one FD=1024 ACTIVATE per key tile (both heads),
  - reciprocal_approx_fast instead of the 8-cycle/elem iterative divide,
  - batch-1 QKV projection and the output projections are emitted at
    query-chunk boundaries as dense PE filler that hides the softmax
    normalizer's DRAM broadcast roundtrip.
"""

import os

import numpy as np

# crash-bisection toggles (default = full kernel)
K_TEST = set(os.environ.get("K_TEST", "").split(","))

B, S, D, H, DK = 2, 2048, 1024, 16, 64
NCORES = 8
HPC = H // NCORES          # heads per core = 2
COLS = HPC * DK            # feature columns per core = 128
T = B * S                  # 4096 tokens
TCH = 512                  # query chunk (fp32 PSUM bank width)
PCH = 1024                 # phase-1 token chunk (f16 moving free dim)
ND = D // 128              # 8 contraction chunks
NE = D // 128              # 8 output-column chunks
NQ = S // TCH              # 4 query chunks per batch
NKT = S // 128             # 16 key tiles per batch
NPC = S // PCH             # 2 phase-1 chunks per batch

_CACHE = {}


def _build_program():
    from contextlib import ExitStack

    import concourse.bacc as bacc
    import concourse.mybir as mybir
    import concourse.tile as tile
    from concourse.masks import make_identity

    f32 = mybir.dt.float32
    f16 = mybir.dt.float16
    Exp = mybir.ActivationFunctionType.Exp

    nc = bacc.Bacc("TRN2", target_bir_lowering=False, debug=False,
                   num_devices=NCORES)

    XT = nc.dram_tensor("xt", [D, T], f16, kind="ExternalInput").ap()
    Wq = nc.dram_tensor("wq", [D, COLS], f16, kind="ExternalInput").ap()
    Wk = nc.dram_tensor("wk", [D, COLS], f16, kind="ExternalInput").ap()
    Wv = nc.dram_tensor("wv", [D, COLS], f16, kind="ExternalInput").ap()
    Bq = nc.dram_tensor("bq", [COLS, 1], f32, kind="ExternalInput").ap()
    Bk = nc.dram_tensor("bk", [COLS, 1], f32, kind="ExternalInput").ap()
    Bv = nc.dram_tensor("bv", [COLS, 1], f32, kind="ExternalInput").ap()
    Wo = nc.dram_tensor("wo", [COLS, D], f16, kind="ExternalInput").ap()
    OT = nc.dram_tensor("ot", [D, T], f16, kind="ExternalOutput").ap()

    import concourse.bass as bass

    with tile.TileContext(nc) as tc, ExitStack() as ctx:
        consts = ctx.enter_context(tc.tile_pool(name="consts", bufs=1))
        xtp = ctx.enter_context(tc.tile_pool(name="xtp", bufs=16))
        big = ctx.enter_context(tc.tile_pool(name="big", bufs=1))
        esbp = ctx.enter_context(tc.tile_pool(name="esbp", bufs=3))
        attnp = ctx.enter_context(tc.tile_pool(name="attnp", bufs=3))
        arp = ctx.enter_context(tc.tile_pool(name="arp", bufs=3))
        rp = ctx.enter_context(tc.tile_pool(name="rp", bufs=2))
        rbp = ctx.enter_context(tc.tile_pool(name="rbp", bufs=4))
        obp = ctx.enter_context(tc.tile_pool(name="obp", bufs=4))
        drp = ctx.enter_context(tc.tile_pool(name="drp", bufs=4, space="DRAM"))
        # PSUM (8 banks): "s" [128,1024]x2 = 4, "pacc" x2 = 2, "po" x2 = 2
        ps_s = ctx.enter_context(tc.tile_pool(name="ps_s", bufs=2, space="PSUM"))
        ps_p = ctx.enter_context(tc.tile_pool(name="ps_p", bufs=2, space="PSUM"))
        ps_o = ctx.enter_context(tc.tile_pool(name="ps_o", bufs=2, space="PSUM"))

        # ---- constants ----
        w_sb = {}
        b_sb = {}
        for nm, src, bsrc in (("wk", Wk, Bk), ("wv", Wv, Bv), ("wq", Wq, Bq)):
            w_sb[nm] = []
            for d in range(ND):
                wt = consts.tile([128, COLS], f16, name=f"{nm}_{d}")
                nc.sync.dma_start(wt, src[d * 128:(d + 1) * 128, :])
                w_sb[nm].append(wt)
            bt = consts.tile([COLS, 1], f32, name=f"b{nm}_sb")
            nc.sync.dma_start(bt, bsrc)
            b_sb[nm] = bt
        wo_sb = []
        for e in range(NE):
            wt = consts.tile([128, 128], f16, name=f"wo_{e}")
            nc.sync.dma_start(wt, Wo[:, e * 128:(e + 1) * 128])
            wo_sb.append(wt)
        onecol = consts.tile([128, 1], f16, name="onecol")
        nc.vector.memset(onecol, 1.0)
        ident = consts.tile([128, 128], f16, name="ident")
        make_identity(nc, ident)
        # pre-load the exp table set so the ~2.7us ACT_TABLE_LOAD happens
        # during phase 1 instead of at the first real exp
        dummy = consts.tile([128, 1], f16, name="dummy")
        nc.scalar.activation(dummy, onecol, Exp)

        QT = big.tile([COLS, T], f16, name="QT")
        KT = big.tile([COLS, T], f16, name="KT")
        proj_out = {"wq": QT, "wk": KT}
        # AV stationaries with the softmax denominator fused in:
        # vpA[b][kt] = [V_headA (64) | ones] -> pacc rows 0:64 + z at row 64
        # vpB[b][kt] = [V_headB (64) | zeros | ones] -> z at row 65, so both
        # z rows land on distinct partitions for one gathered reciprocal
        vpA = [[None] * NKT for _ in range(B)]
        vpB = [[None] * NKT for _ in range(B)]
        for b in range(B):
            for kt in range(NKT):
                vpA[b][kt] = consts.tile([128, DK + 1], f16,
                                         name=f"vpA_{b}_{kt}")
                # 32 zero columns pad head B's z row out to partition 96,
                # the next 32-aligned base the vector engine can address
                vpB[b][kt] = consts.tile([128, DK + 33], f16,
                                         name=f"vpB_{b}_{kt}")
                nc.vector.memset(vpA[b][kt][:, DK:DK + 1], 1.0)
                nc.vector.memset(vpB[b][kt][:, DK:DK + 32], 0.0)
                nc.vector.memset(vpB[b][kt][:, DK + 32:DK + 33], 1.0)

        xts = {}

        def emit_x_dma(b, t):
            ts = slice(b * S + t * PCH, b * S + (t + 1) * PCH)
            cur = []
            for d in range(ND):
                xt_t = xtp.tile([128, PCH], f16, name=f"xt_{b}_{t}_{d}",
                                tag="xt")
                nc.gpsimd.dma_start(xt_t, XT[d * 128:(d + 1) * 128, ts])
                cur.append(xt_t)
            xts[(b, t)] = cur

        def emit_proj_unit(b, t, nm):
            """One phase-1 unit: [128, PCH] of one projection for batch b."""
            ts = slice(b * S + t * PCH, b * S + (t + 1) * PCH)
            acc = ps_s.tile([128, PCH], f32, tag="s", name=f"acc_{b}_{t}_{nm}")
            for half in range(2):
                hs = slice(half * TCH, (half + 1) * TCH)
                for d in range(ND):
                    nc.tensor.matmul(acc[:, hs], lhsT=w_sb[nm][d],
                                     rhs=xts[(b, t)][d][:, hs],
                                     start=(d == 0), stop=(d == ND - 1))
            if nm == "wv":
                vt = big.tile([COLS, PCH], f16, tag="vt", bufs=2,
                              name=f"vt_{b}_{t}")
                nc.vector.tensor_scalar_add(vt, acc, b_sb[nm])
                vts[(b, t)] = vt
            else:
                nc.vector.tensor_scalar_add(proj_out[nm][:, ts], acc, b_sb[nm])

        vts = {}

        # ---- emission ----
        # PSUM budget (8 banks): tag "s" [128,1024]x2 = 4 (scores pairs +
        # phase-1 accumulators), tag "pacc" [128,512]x2 = 2 (combined AV
        # accumulator per q-chunk, double-buffered), tag "z" x1 = 1 (both
        # softmax-denominator rows), tag "po" x1 = 1 (outproj + V-transposes).
        #
        # Head A's kt0 matmul carries start=True and clears the whole bank;
        # head B's kt0 matmul uses start=False and lands on cleared
        # has_written bits, so it overwrites rather than accumulates stale
        # data. Emission order (A before B at kt0) is load-bearing.
        attn_of = {}
        araw_of = {}
        pacc_of = {}
        z_of = {}
        rb_of = {}

        def emit_attention_chunk(b, q, fillers=()):
            qs = slice(b * S + q * TCH, b * S + (q + 1) * TCH)
            paccA = ps_p.tile([128, TCH], f32, tag="pacc", name=f"pA_{b}_{q}")
            paccB = ps_p.tile([128, TCH], f32, tag="pacc", name=f"pB_{b}_{q}")
            pacc_of[(b, q)] = (paccA, paccB)
            fillers = list(fillers)
            fi = 0
            sprev = None
            for kt in range(NKT + 1):
                if kt < NKT:
                    ks = slice(b * S + kt * 128, b * S + (kt + 1) * 128)
                    s = ps_s.tile([128, 2 * TCH], f32, tag="s",
                                  name=f"s_{b}_{q}_{kt}")
                    nc.tensor.matmul(s[:, 0:TCH], lhsT=KT[0:DK, ks],
                                     rhs=QT[0:DK, qs], start=True, stop=True)
                    nc.tensor.matmul(s[:, TCH:2 * TCH], lhsT=KT[DK:2 * DK, ks],
                                     rhs=QT[DK:2 * DK, qs],
                                     start=True, stop=True)
                    esb = esbp.tile([128, 2 * TCH], f16, tag="esb",
                                    name=f"esb_{b}_{q}_{kt}")
                    # one ACTIVATE per PSUM bank — a single FD=1024 read
                    # crossing the bank boundary is fatal on hardware
                    nc.scalar.activation(esb[:, 0:TCH], s[:, 0:TCH],
                                         Exp, scale=0.125)
                    nc.scalar.activation(esb[:, TCH:2 * TCH],
                                         s[:, TCH:2 * TCH],
                                         Exp, scale=0.125)
                if sprev is not None:
                    k0, esb0 = sprev
                    st, sp = (k0 == 0), (k0 == NKT - 1)
                    nc.tensor.matmul(paccA[0:DK + 1, :], lhsT=vpA[b][k0],
                                     rhs=esb0[:, 0:TCH], start=st, stop=sp)
                    nc.tensor.matmul(paccB[0:DK + 33, :], lhsT=vpB[b][k0],
                                     rhs=esb0[:, TCH:2 * TCH],
                                     start=st, stop=sp)
                    if k0 >= 1 and fi < len(fillers):
                        fillers[fi]()
                        fi += 1
                if kt < NKT:
                    sprev = (kt, esb)
            while fi < len(fillers):
                fillers[fi]()
                fi += 1

        def emit_muls(bq):
            """Normalize staged attention with the broadcast reciprocals."""
            ar = araw_of[bq]
            rbA, rbB = rb_of[bq]
            attn = attnp.tile([128, TCH], f16, tag="attn",
                              name=f"at_{bq[0]}_{bq[1]}")
            nc.vector.tensor_mul(attn[0:DK, :], ar[0:DK, :], rbA[0:DK, :])
            nc.vector.tensor_mul(attn[64:128, :], ar[64:128, :],
                                 rbB[64:128, :])
            attn_of[bq] = attn

        def emit_epilogue(b, q, prev=None):
            """Stage pacc/z out of PSUM (freeing the banks fast), normalize
            the previous chunk, and launch this chunk's reciprocal +
            DRAM-broadcast roundtrip."""
            paccA, paccB = pacc_of[(b, q)]
            ar = arp.tile([128, TCH], f16, tag="araw", name=f"ar_{b}_{q}")
            art = arp.tile([128, TCH], f16, tag="arawB", bufs=2,
                           name=f"art_{b}_{q}")
            nc.vector.tensor_copy(ar[0:DK, :], paccA[0:DK, :])
            nc.vector.tensor_copy(art[0:DK, :], paccB[0:DK, :])
            zg = rp.tile([128, TCH], f32, tag="zg", name=f"zg_{b}_{q}")
            nc.vector.memset(zg[64:96, :], 1.0)
            nc.vector.tensor_copy(zg[64:65, :], paccA[64:65, :])
            nc.vector.tensor_copy(zg[96:97, :], paccB[96:97, :])
            araw_of[(b, q)] = ar
            # head B's rows live at partitions 0:64; shift them to 64:128
            # through a DRAM bounce (DMA moves across partitions, DVE cannot)
            arsc = drp.tile([DK, TCH], f16, tag="ars", name=f"ars_{b}_{q}")
            nc.gpsimd.dma_start(arsc, art[0:DK, :])
            nc.gpsimd.dma_start(ar[64:128, :], arsc)
            if prev is not None:
                emit_muls(prev)
            rg = rp.tile([128, TCH], f32, tag="r", name=f"rg_{b}_{q}")
            nc.vector.reciprocal(rg[64:97, :], zg[64:97, :])
            scrA = drp.tile([1, TCH], f32, tag="scr", name=f"scrA_{b}_{q}")
            scrB = drp.tile([1, TCH], f32, tag="scr", name=f"scrB_{b}_{q}")
            nc.gpsimd.dma_start(scrA, rg[64:65, :])
            nc.gpsimd.dma_start(scrB, rg[96:97, :])
            rbA = rbp.tile([128, TCH], f32, tag="rb", name=f"rbA_{b}_{q}")
            rbB = rbp.tile([128, TCH], f32, tag="rb", name=f"rbB_{b}_{q}")
            nc.gpsimd.dma_start(
                rbA[0:DK, :],
                bass.AP(tensor=scrA.tensor, offset=scrA.offset,
                        ap=[[0, DK]] + scrA.ap[1:]))
            nc.gpsimd.dma_start(
                rbB[64:128, :],
                bass.AP(tensor=scrB.tensor, offset=scrB.offset,
                        ap=[[0, DK]] + scrB.ap[1:]))
            rb_of[(b, q)] = (rbA, rbB)

        def mk_po(b, q, e):
            def f():
                attn = attn_of[(b, q)]
                qs = slice(b * S + q * TCH, b * S + (q + 1) * TCH)
                po = ps_o.tile([128, TCH], f32, tag="po",
                               name=f"po_{b}_{q}_{e}")
                nc.tensor.matmul(po, lhsT=wo_sb[e], rhs=attn,
                                 start=True, stop=True)
                ob = obp.tile([128, TCH], f16, tag="ob",
                              name=f"ob_{b}_{q}_{e}")
                nc.vector.tensor_copy(ob, po)
                nc.sync.dma_start(OT[e * 128:(e + 1) * 128, qs], ob)
            return f

        def po_fillers(b, q):
            return [mk_po(b, q, e) for e in range(NE)]

        def mk_proj(b, t, nm):
            return lambda: emit_proj_unit(b, t, nm)

        def mk_tp(b, ktiles):
            def f():
                for kt in ktiles:
                    t = (kt * 128) // PCH
                    c0 = (kt * 128) % PCH
                    ptrz = ps_o.tile([128, TCH], f32, tag="po",
                                     name=f"ptr_{b}_{kt}")
                    ptr = ptrz.bitcast(f16)[:, 0:128]
                    nc.tensor.transpose(ptr, vts[(b, t)][:, c0:c0 + 128],
                                        ident)
                    nc.vector.tensor_copy(vpA[b][kt][:, 0:DK], ptr[:, 0:DK])
                    nc.vector.tensor_copy(vpB[b][kt][:, 0:DK],
                                          ptr[:, DK:2 * DK])
            return f

        def mk_xdma(b):
            def f():
                for t in range(NPC):
                    emit_x_dma(b, t)
            return f

        # phase-1 head: only what batch-0 q-chunk 0 needs up front; the rest
        # streams in as kt-loop fillers so the PE never idles and the scalar
        # engine starts exping as early as possible
        for t in range(NPC):
            emit_x_dma(0, t)
        emit_proj_unit(0, 0, "wk")
        emit_proj_unit(0, 0, "wq")
        emit_proj_unit(0, 0, "wv")
        mk_tp(0, range(0, 8))()

        emit_attention_chunk(0, 0, [
            mk_proj(0, 1, "wk"), mk_proj(0, 1, "wv"),
            mk_tp(0, [8, 9]), mk_tp(0, [10, 11]),
            mk_tp(0, [12, 13]), mk_tp(0, [14, 15]),
        ])
        emit_epilogue(0, 0)

        emit_attention_chunk(0, 1, [
            mk_proj(0, 1, "wq"), mk_xdma(1),
            mk_proj(1, 0, "wv"),
        ])
        emit_epilogue(0, 1, prev=(0, 0))

        emit_attention_chunk(0, 2, po_fillers(0, 0) + [
            mk_proj(1, 1, "wv"),
            mk_tp(1, [0, 1]), mk_tp(1, [2, 3]),
        ])
        emit_epilogue(0, 2, prev=(0, 1))

        emit_attention_chunk(0, 3, po_fillers(0, 1) + [
            mk_tp(1, [4, 5]), mk_tp(1, [6, 7]),
            mk_proj(1, 0, "wk"), mk_proj(1, 1, "wk"),
            mk_proj(1, 0, "wq"),
        ])
        emit_epilogue(0, 3, prev=(0, 2))

        emit_attention_chunk(1, 0, [
            mk_tp(1, [8, 9]), mk_tp(1, [10, 11]),
            mk_tp(1, [12, 13]), mk_tp(1, [14, 15]),
        ] + po_fillers(0, 2))
        emit_epilogue(1, 0, prev=(0, 3))

        emit_attention_chunk(1, 1, [mk_proj(1, 1, "wq")]
                             + po_fillers(0, 3))
        emit_epilogue(1, 1, prev=(1, 0))

        emit_attention_chunk(1, 2, po_fillers(1, 0))
        emit_epilogue(1, 2, prev=(1, 1))

        emit_attention_chunk(1, 3, po_fillers(1, 1))
        emit_epilogue(1, 3, prev=(1, 2))

        for f in po_fillers(1, 2):
            f()
        emit_muls((1, 3))
        for f in po_fillers(1, 3):
            f()

    nc.compile()
    return nc


def _get_program():
    if "nc" not in _CACHE:
        _CACHE["nc"] = _build_program()
    return _CACHE["nc"]


def _install_ntff_hook():
    """Provide the antenv.axon_hooks shim this container's antenv lacks so
    run_bass_kernel_spmd(trace=True) can capture NTFF profiles."""
    import sys
    import types

    try:
        import antenv

        if hasattr(antenv, "axon_hooks"):
            return
        mod = types.ModuleType("antenv.axon_hooks")
        mod._hook = None
        mod.set_axon_ntff_profile_hook = lambda h: setattr(mod, "_hook", h)
        mod.get_axon_ntff_profile_hook = lambda: mod._hook
        sys.modules["antenv.axon_hooks"] = mod
        antenv.axon_hooks = mod
        from trn_agent_boot.trn_boot import _ntff_profile_via_ctypes

        mod.set_axon_ntff_profile_hook(
            _ntff_profile_via_ctypes("/opt/axon/libaxon_pjrt.so"))
    except Exception:
        pass


def kernel(X, Wq, bq, Wk, bk, Wv, bv, Wo, bo, _profile=False, _trace_cores=None):
    from concourse.bass_utils import run_bass_kernel_spmd

    if _profile:
        _install_ntff_hook()

    nc = _get_program()

    XT = np.ascontiguousarray(
        np.asarray(X, np.float32).reshape(T, D).T).astype(np.float16)
    Wq, Wk, Wv, Wo = (np.asarray(w, np.float16) for w in (Wq, Wk, Wv, Wo))
    bq, bk, bv = (np.asarray(v, np.float32) for v in (bq, bk, bv))
    bo = np.asarray(bo, np.float32)

    in_maps = []
    for c in range(NCORES):
        cs = slice(c * COLS, (c + 1) * COLS)
        in_maps.append({
            "xt": XT,
            "wq": np.ascontiguousarray(Wq[:, cs]),
            "wk": np.ascontiguousarray(Wk[:, cs]),
            "wv": np.ascontiguousarray(Wv[:, cs]),
            "bq": np.ascontiguousarray(bq[cs].reshape(COLS, 1)),
            "bk": np.ascontiguousarray(bk[cs].reshape(COLS, 1)),
            "bv": np.ascontiguousarray(bv[cs].reshape(COLS, 1)),
            "wo": np.ascontiguousarray(Wo[cs, :]),
        })

    res = run_bass_kernel_spmd(
        nc, in_maps, core_ids=list(range(NCORES)),
        trace=_profile,
        trace_cores=(_trace_cores if _trace_cores is not None
                     else ([0] if _profile else None)),
    )

    ot = res.results[0]["ot"].astype(np.float32)
    for c in range(1, NCORES):
        ot += res.results[c]["ot"].astype(np.float32)
    out = (ot.T + bo).astype(np.float32).reshape(B, S, D)
    if _profile:
        kernel.last_exec_time_ns = res.exec_time_ns
        kernel.last_results = res
    return out
